# revision 13
# baseline (speedup 1.0000x reference)
"""Trainium2 distributed kernel: 4-layer attention encoder (B=4, D=1024, H=16, N=1024).

Sharding: (batch, sequence-half) across 8 NeuronCores - core r owns batch
b = r//2 and sequence half r%2 (512 columns). All conv1x1 projections and
the MLP are per-column -> fully local. Per layer each core computes its
K / V^T shard and AllGathers it with its batch peer only (2-rank groups),
then runs attention for its 512 query columns of its batch.

Attention layout (v2): scores are computed TRANSPOSED - scT[m, n] with
keys m on partitions (lhsT = K-block, rhs = Q streaming 512 wide). exp runs
on the scalar engine with no accumulate and lands bf16 weights directly in
the [key, query] layout the PV matmul consumes - no DMA transposes at all.
The PV matmul keeps V^T blocks stationary with an appended ones-column so
the softmax denominator drops out of the same accumulation (extra PSUM row);
normalization = reciprocal_approx_fast on the denominator row, a rank-1
ones x rinv broadcast matmul, and one tensor_tensor multiply.

Host-side preprocessing (exact, fp32):
  - channel permutation to head-major so each head's 64 channels are contiguous
  - 1/sqrt(DK) folded into Wq/bq
  - bk dropped (constant-per-row shift is softmax invariant)
  - bv folded into the merge bias (softmax rows sum to 1): bm_eff = bm + Wm @ bv
  - BatchNorm (eval) + p1 bias folded to per-channel scale/bias applied in the
    Relu activation: h = relu(s1 * p1_raw + b1)
  - streamed lhsT weights packed per output-tile so every weight DMA is one
    contiguous block

Compute dtype: bf16 matmul inputs, fp32 PSUM accumulation; the residual
stream is bf16 (requantized once per layer by the residual add).
"""

import numpy as np
import ml_dtypes

import concourse.bass as bass
import concourse.mybir as mybir
import concourse.tile as tile
from concourse import bacc
from concourse.bass_utils import run_bass_kernel_spmd

L, D, H, B, N = 4, 1024, 16, 4, 1024
DK = D // H          # 64
R = 8                # cores
NS = N // 2          # 512 per-core sequence columns (one batch, half sequence)
DT = D // 128        # 8 d-tiles
NT = NS // 128       # 4 n-tiles per core
VW = 192             # per head-pair block width in the augmented V^T tiles
BF = mybir.dt.bfloat16
F32 = mybir.dt.float32
BFNP = ml_dtypes.bfloat16

# head-major channel permutation: perm[h*64+dk] = dk*16+h
PERM = np.array([dk * H + h for h in range(H) for dk in range(DK)])


def _wtile_stream(w_t):
    """(C, M) weight -> (M//128, 128, C//128*128): arr[mt, p, ct*128+mo] =
    w_t[ct*128+p, mt*128+mo]. Each [mt] block is one contiguous lhsT tile."""
    c, m = w_t.shape
    a = w_t.reshape(c // 128, 128, m // 128, 128)      # (ct, p, mt, mo)
    a = a.transpose(2, 1, 0, 3)                        # (mt, p, ct, mo)
    return np.ascontiguousarray(a.reshape(m // 128, 128, -1)).astype(BFNP)


def _wtile_res(w_t):
    """(C, M) weight -> (128, C//128*M) [p, ct*M + m] for resident rhs use."""
    c, m = w_t.shape
    return np.ascontiguousarray(
        w_t.reshape(c // 128, 128, m).transpose(1, 0, 2).reshape(128, -1)
    ).astype(BFNP)


def _btile(b_vec):
    """(C,) bias -> (128, C//128) [p, ct]."""
    c = b_vec.shape[0]
    return np.ascontiguousarray(b_vec.reshape(c // 128, 128).T).astype(np.float32)


def prepare_host_inputs(inputs):
    """Preprocess full weights once; returns dict of shard-independent arrays."""
    Wq, bq = inputs["Wq"], inputs["bq"]
    Wk = inputs["Wk"]
    Wv, bv = inputs["Wv"], inputs["bv"]
    Wm, bm = inputs["Wm"], inputs["bm"]
    Wp1, bp1 = inputs["Wp1"], inputs["bp1"]
    g, beta = inputs["bn_gamma"], inputs["bn_beta"]
    mu, var = inputs["bn_mean"], inputs["bn_var"]
    Wp2 = inputs["Wp2"]

    out = {k: [] for k in ("wq", "wk", "wv", "wm", "wp1", "wp2", "bq", "bm", "s1", "b1")}
    for l in range(L):
        out["wq"].append(_wtile_stream((Wq[l][PERM] / 8.0).T))
        out["wk"].append(_wtile_stream(Wk[l][PERM].T))
        out["wv"].append(_wtile_res(Wv[l][PERM].T))
        out["wm"].append(_wtile_stream(Wm[l][:, PERM].T))
        out["wp1"].append(_wtile_stream(Wp1[l].T))
        out["wp2"].append(_wtile_stream(Wp2[l].T))
        out["bq"].append(_btile(bq[l][PERM] / 8.0))
        bm_eff = bm[l] + Wm[l] @ bv[l]
        out["bm"].append(_btile(bm_eff))
        s1 = g[l] / np.sqrt(var[l] + 1e-5)
        b1 = beta[l] + s1 * (bp1[l] - mu[l])
        out["s1"].append(_btile(s1))
        out["b1"].append(_btile(b1))
    res = {k: np.stack(v) for k, v in out.items()}
    # biases: (L, 128, C) -> (128, L*C) so the device DMA is a plain copy
    for k in ("bq", "bm", "s1", "b1"):
        res[k] = np.ascontiguousarray(res[k].transpose(1, 0, 2).reshape(128, -1))
    return res


def shard_x(motion_feats, r):
    """(B, D, N) -> core r's (128, DT*NS) bf16 tile layout [p, ct*NS + n]."""
    b, half = r // 2, r % 2
    m = motion_feats[b, :, half * NS : (half + 1) * NS]    # (D, NS)
    m = m.reshape(DT, 128, NS).transpose(1, 0, 2)          # (p, ct, n)
    return np.ascontiguousarray(m.reshape(128, DT * NS)).astype(BFNP)


def unshard_out(res_list):
    """8 x (128, DT*NS) -> (B, D, N)."""
    out = np.empty((B, D, N), dtype=np.float32)
    for r, arr in enumerate(res_list):
        b, half = r // 2, r % 2
        m = arr.reshape(128, DT, NS).transpose(1, 0, 2)    # (ct, p, n)
        out[b, :, half * NS : (half + 1) * NS] = m.reshape(D, NS)
    return out


def build_nc():
    nc = bacc.Bacc("TRN2", target_bir_lowering=False, debug=False, num_devices=R)

    x_in = nc.dram_tensor("x_in", [128, DT * NS], BF, kind="ExternalInput")
    wq = nc.dram_tensor("wq", [L, DT, 128, D], BF, kind="ExternalInput")
    wk = nc.dram_tensor("wk", [L, DT, 128, D], BF, kind="ExternalInput")
    wv = nc.dram_tensor("wv", [L, 128, DT * D], BF, kind="ExternalInput")
    wm = nc.dram_tensor("wm", [L, DT, 128, D], BF, kind="ExternalInput")
    wp1 = nc.dram_tensor("wp1", [L, 16, 128, 2048], BF, kind="ExternalInput")
    wp2 = nc.dram_tensor("wp2", [L, DT, 128, 2048], BF, kind="ExternalInput")
    bq_d = nc.dram_tensor("bq", [128, L * 8], F32, kind="ExternalInput")
    bm_d = nc.dram_tensor("bm", [128, L * 8], F32, kind="ExternalInput")
    s1_d = nc.dram_tensor("s1", [128, L * 16], F32, kind="ExternalInput")
    b1_d = nc.dram_tensor("b1", [128, L * 16], F32, kind="ExternalInput")
    out_e = nc.dram_tensor("out", [128, DT * NS], F32, kind="ExternalOutput")
    DBG = False
    if DBG:
        dbg_k = nc.dram_tensor("dbg_k", [128, DT * NS], BF, kind="ExternalOutput")
        dbg_q = nc.dram_tensor("dbg_q", [128, DT * NS], BF, kind="ExternalOutput")
        dbg_kt = nc.dram_tensor("dbg_kt", [128, N], BF, kind="ExternalOutput")
        dbg_va = nc.dram_tensor("dbg_va", [128, DT * 192], BF, kind="ExternalOutput")
        dbg_ex = nc.dram_tensor("dbg_ex", [2, 128, DT * NS], BF, kind="ExternalOutput")
        dbg_atb = nc.dram_tensor("dbg_atb", [2, 128, NS], BF, kind="ExternalOutput")
        dbg_rin = nc.dram_tensor("dbg_rin", [2, 128, NS], F32, kind="ExternalOutput")
        dbg_attn = nc.dram_tensor("dbg_attn", [128, DT * NS], BF, kind="ExternalOutput")
        dbg_h1 = nc.dram_tensor("dbg_h1", [128, 16 * NS], BF, kind="ExternalOutput")
        dbg_x1 = nc.dram_tensor("dbg_x1", [L, 128, DT * NS], BF, kind="ExternalOutput")

    ADD = mybir.AluOpType.add
    MULT = mybir.AluOpType.mult
    AF = mybir.ActivationFunctionType
    GROUPS = [[0, 1], [2, 3], [4, 5], [6, 7]]

    with tile.TileContext(nc) as tc:
        with (
            tc.tile_pool(name="const", bufs=1) as const,
            tc.tile_pool(name="acts", bufs=1) as acts,
            tc.tile_pool(name="kv", bufs=1) as kvp,
            tc.tile_pool(name="wstr", bufs=3) as wstr,
            tc.tile_pool(name="wres", bufs=1) as wres,
            tc.tile_pool(name="expool", bufs=3) as expool,
            tc.tile_pool(name="small", bufs=2) as smp,
            tc.tile_pool(name="pp", bufs=2, space="PSUM") as ppp,
            tc.tile_pool(name="sc", bufs=2, space="PSUM") as scp,
            tc.tile_pool(name="at", bufs=2, space="PSUM") as atp,
            tc.tile_pool(name="dram", bufs=2, space="DRAM") as dramp,
        ):
            bq_sb = const.tile([128, L * 8], F32)
            nc.sync.dma_start(bq_sb[:], bq_d[:, :])
            bm_sb = const.tile([128, L * 8], F32)
            nc.sync.dma_start(bm_sb[:], bm_d[:, :])
            s1_sb = const.tile([128, L * 16], F32)
            nc.sync.dma_start(s1_sb[:], s1_d[:, :])
            b1_sb = const.tile([128, L * 16], F32)
            nc.sync.dma_start(b1_sb[:], b1_d[:, :])
            ones_sb = const.tile([128, 64], BF)
            nc.vector.memset(ones_sb[:], 1.0)

            x_bf = acts.tile([128, DT * NS], BF)
            nc.sync.dma_start(x_bf[:], x_in[:, :])
            q_bf = acts.tile([128, DT * NS], BF)
            attn_bf = acts.tile([128, DT * NS], BF)
            mg_bf = acts.tile([128, DT * NS], BF)
            h1_bf = acts.tile([128, 16 * NS], BF)
            k_sh = acts.tile([128, DT * NS], BF)
            v_sh = acts.tile([128, NT * D], BF)

            # persistent gathered-KV tiles; the augmented V^T tiles carry a
            # ones column (softmax denominator) and a zero gap per head pair:
            # pair t block [vA(64) | ones(1) | 0(63) | vB(64)] at t*VW
            kts = []
            for t in range(DT):
                kt = kvp.tile([128, N], BF, tag=f"kt{t}", name=f"kt{t}")
                kts.append(kt)
            vas = []
            for r in range(DT):
                va = kvp.tile([128, DT * VW], BF, tag=f"va{r}", name=f"va{r}")
                nc.vector.memset(va[:], 0.0)
                nc.vector.memset(
                    va[:].rearrange("p (t c) -> p t c", c=VW)[:, :, 64:65], 1.0
                )
                vas.append(va)

            def stream_w(src, l, mt, tag, eng=None):
                """One contiguous lhsT m-tile: all contraction chunks for mt.
                The bulk MLP streams go via SWDGE (gpsimd) so they cannot
                head-of-line-block the projection weights on the sync ring."""
                t = wstr.tile([128, src.shape[3]], BF, tag=tag, name="w_t")
                (eng or nc.sync).dma_start(t[:], src[l, mt, :, :])
                return t

            for l in range(L):
                # ---- K projection (feeds the collective first) ----
                for mt in range(DT):
                    w_t = stream_w(wk, l, mt, "w1k")
                    ps = ppp.tile([128, NS], F32, tag="pp")
                    for ct in range(DT):
                        nc.tensor.matmul(
                            ps[:],
                            w_t[:, ct * 128 : (ct + 1) * 128],
                            x_bf[:, ct * NS : (ct + 1) * NS],
                            start=(ct == 0),
                            stop=(ct == DT - 1),
                        )
                    nc.vector.tensor_copy(k_sh[:, mt * NS : (mt + 1) * NS], ps[:])
                ck_i = dramp.tile([128, DT * NS], BF, tag="cki")
                nc.scalar.dma_start(ck_i[:, :], k_sh[:])
                ck_o = dramp.tile([2 * 128, DT * NS], BF, tag="cko")
                nc.gpsimd.collective_compute(
                    "AllGather",
                    mybir.AluOpType.bypass,
                    replica_groups=GROUPS,
                    ins=[ck_i[:].opt()],
                    outs=[ck_o[:].opt()],
                )

                # ---- V^T projection: out[n, d], n-tiles of 128 ----
                wv_sb = wres.tile([128, DT * D], BF, tag="wv", name="wv_sb")
                nc.scalar.dma_start(wv_sb[:], wv[l, :, :])
                for nt in range(NT):
                    for dh in range(2):
                        ps = ppp.tile([128, NS], F32, tag="pp")
                        for ct in range(DT):
                            nc.tensor.matmul(
                                ps[:],
                                x_bf[:, ct * NS + nt * 128 : ct * NS + (nt + 1) * 128],
                                wv_sb[:, ct * D + dh * 512 : ct * D + (dh + 1) * 512],
                                start=(ct == 0),
                                stop=(ct == DT - 1),
                            )
                        nc.vector.tensor_copy(
                            v_sh[:, nt * D + dh * 512 : nt * D + (dh + 1) * 512], ps[:]
                        )
                cv_i = dramp.tile([128, NT * D], BF, tag="cvi")
                nc.scalar.dma_start(cv_i[:, :], v_sh[:])
                cv_o = dramp.tile([2 * 128, NT * D], BF, tag="cvo")
                nc.gpsimd.collective_compute(
                    "AllGather",
                    mybir.AluOpType.bypass,
                    replica_groups=GROUPS,
                    ins=[cv_i[:].opt()],
                    outs=[cv_o[:].opt()],
                )

                # ---- Q projection (+bias, 1/8 prefolded) ----
                for mt in range(DT):
                    w_t = stream_w(wq, l, mt, "w1k")
                    ps = ppp.tile([128, NS], F32, tag="pp")
                    for ct in range(DT):
                        nc.tensor.matmul(
                            ps[:],
                            w_t[:, ct * 128 : (ct + 1) * 128],
                            x_bf[:, ct * NS : (ct + 1) * NS],
                            start=(ct == 0),
                            stop=(ct == DT - 1),
                        )
                    nc.vector.tensor_scalar_add(
                        q_bf[:, mt * NS : (mt + 1) * NS],
                        ps[:],
                        bq_sb[:, l * 8 + mt : l * 8 + mt + 1],
                    )

                # ---- gathered K/V loads ----
                # ck_o rows r2*128 + p: K shard [p, mt*NS+ns] (d = mt*128+p)
                # cv_o rows r2*128 + p: V^T shard [p, nt*D+d] (m = r2*NS + nt*128 + p)
                ko = ck_o[:].rearrange("(r p) (mt ns) -> r p mt ns", r=2, mt=DT)
                for t in range(DT):
                    nc.scalar.dma_start(
                        kts[t][:].rearrange("p (r ns) -> p r ns", r=2),
                        ko[:, :, t, :].rearrange("r p ns -> p r ns"),
                    )
                vo = cv_o[:].rearrange("(r p) (nt d) -> r p nt d", r=2, nt=NT)
                for r in range(DT):
                    s4 = vo[r // NT, :, r % NT, :].rearrange(
                        "p (t a d) -> p t a d", a=2, d=64
                    )
                    dst = vas[r][:].rearrange("p (t c) -> p t c", c=VW)
                    nc.scalar.dma_start(dst[:, :, 0:64], s4[:, :, 0, :])
                    nc.scalar.dma_start(dst[:, :, 128:192], s4[:, :, 1, :])

                if DBG and l == 0:
                    nc.sync.dma_start(dbg_k[:, :], k_sh[:])
                    nc.sync.dma_start(dbg_q[:, :], q_bf[:])
                    nc.sync.dma_start(dbg_kt[:, :], kts[0][:])
                    nc.sync.dma_start(dbg_va[:, :], vas[0][:])

                # ---- attention: transposed scores, per head-pair tile t ----
                for t in range(DT):
                    exs = [
                        expool.tile([128, DT * NS], BF, tag="ex", name="ex")
                        for _ in range(2)
                    ]
                    for mtp in range(4):
                        for hi in range(2):
                            Hs = slice(64 * hi, 64 * (hi + 1))
                            sc = scp.tile([128, N], F32, tag="sc")
                            for sub in range(2):
                                mt = 2 * mtp + sub
                                nc.tensor.matmul(
                                    sc[:, sub * NS : (sub + 1) * NS],
                                    kts[t][Hs, mt * 128 : (mt + 1) * 128],
                                    q_bf[Hs, t * NS : (t + 1) * NS],
                                    start=True,
                                    stop=True,
                                )
                            nc.scalar.activation(
                                exs[hi][:, mtp * N : (mtp + 1) * N], sc[:], AF.Exp
                            )
                    at_bfs = []
                    for hi in range(2):
                        base = t * VW
                        at = atp.tile([128, NS], F32, tag="at")
                        for r in range(DT):
                            lhsT = (
                                vas[r][:, base : base + 65]
                                if hi == 0
                                else vas[r][:, base + 64 : base + VW]
                            )
                            out_ap = at[0:65, :] if hi == 0 else at[:, :]
                            nc.tensor.matmul(
                                out_ap,
                                lhsT,
                                exs[hi][:, r * NS : (r + 1) * NS],
                                start=(r == 0),
                                stop=(r == DT - 1),
                            )
                        # copy v-rows AND the denominator row to bf16 SBUF
                        at_bf = smp.tile([128, NS], BF, tag="atbf", name="at_bf", bufs=3)
                        if hi == 0:
                            nc.vector.tensor_copy(at_bf[0:65, :], at[0:65, :])
                        else:
                            nc.vector.tensor_copy(at_bf[:, :], at[:, :])
                        at_bfs.append(at_bf)
                    # broadcast both heads' denominator rows into one PSUM tile
                    # (rank-1 matmuls), then a single reciprocal_approx_fast on
                    # the full 128 partitions - the custom DVE op silently
                    # no-ops when its AP base partition is nonzero.
                    rb = ppp.tile([128, NS], F32, tag="pp")
                    nc.tensor.matmul(
                        rb[0:64, :],
                        ones_sb[64:65, 0:64],
                        at_bfs[0][64:65, :],
                        start=True,
                        stop=True,
                    )
                    nc.tensor.matmul(
                        rb[64:128, :],
                        ones_sb[0:1, 0:64],
                        at_bfs[1][0:1, :],
                        start=True,
                        stop=True,
                    )
                    rb_sb = smp.tile([128, NS], F32, tag="rbsb", name="rb_sb")
                    nc.vector.tensor_copy(rb_sb[:, :], rb[:, :])
                    rinv_sb = smp.tile([128, NS], F32, tag="rinv", name="rinv_sb")
                    nc.vector.reciprocal_approx_fast(rinv_sb[:, :], rb_sb[:, :])
                    for hi in range(2):
                        Hs = slice(64 * hi, 64 * (hi + 1))
                        nc.vector.tensor_tensor(
                            attn_bf[Hs, t * NS : (t + 1) * NS],
                            at_bfs[hi][Hs, :],
                            rinv_sb[Hs, :],
                            op=MULT,
                        )
                    if DBG and l == 0 and t == 0:
                        for hi in range(2):
                            nc.sync.dma_start(dbg_ex[hi, :, :], exs[hi][:])
                            nc.sync.dma_start(dbg_atb[hi, :, :], at_bfs[hi][:])
                            nc.sync.dma_start(dbg_rin[hi, :, :], rinv_sb[:])

                if DBG and l == 0:
                    nc.sync.dma_start(dbg_attn[:, :], attn_bf[:])

                # ---- merge ----
                for mt in range(DT):
                    w_t = stream_w(wm, l, mt, "w1k")
                    ps = ppp.tile([128, NS], F32, tag="pp")
                    for ct in range(DT):
                        nc.tensor.matmul(
                            ps[:],
                            w_t[:, ct * 128 : (ct + 1) * 128],
                            attn_bf[:, ct * NS : (ct + 1) * NS],
                            start=(ct == 0),
                            stop=(ct == DT - 1),
                        )
                    nc.vector.tensor_scalar_add(
                        mg_bf[:, mt * NS : (mt + 1) * NS],
                        ps[:],
                        bm_sb[:, l * 8 + mt : l * 8 + mt + 1],
                    )

                # ---- p1 + BN + relu (contraction: 8 merged chunks + 8 x chunks) ----
                for mt in range(16):
                    w_t = stream_w(wp1, l, mt, "w2k", nc.gpsimd)
                    ps = ppp.tile([128, NS], F32, tag="pp")
                    for ct in range(16):
                        rhs = (
                            mg_bf[:, ct * NS : (ct + 1) * NS]
                            if ct < 8
                            else x_bf[:, (ct - 8) * NS : (ct - 7) * NS]
                        )
                        nc.tensor.matmul(
                            ps[:],
                            w_t[:, ct * 128 : (ct + 1) * 128],
                            rhs,
                            start=(ct == 0),
                            stop=(ct == 15),
                        )
                    nc.scalar.activation(
                        h1_bf[:, mt * NS : (mt + 1) * NS],
                        ps[:],
                        AF.Relu,
                        bias=b1_sb[:, l * 16 + mt : l * 16 + mt + 1],
                        scale=s1_sb[:, l * 16 + mt : l * 16 + mt + 1],
                    )

                # ---- p2 + residual (bf16 stream, in-place) ----
                for ot in range(DT):
                    w_t = stream_w(wp2, l, ot, "w2k", nc.gpsimd)
                    ps = ppp.tile([128, NS], F32, tag="pp")
                    for ct in range(16):
                        nc.tensor.matmul(
                            ps[:],
                            w_t[:, ct * 128 : (ct + 1) * 128],
                            h1_bf[:, ct * NS : (ct + 1) * NS],
                            start=(ct == 0),
                            stop=(ct == 15),
                        )
                    nc.vector.tensor_tensor(
                        x_bf[:, ot * NS : (ot + 1) * NS],
                        x_bf[:, ot * NS : (ot + 1) * NS],
                        ps[:],
                        op=ADD,
                    )

                if DBG and l == 0:
                    nc.sync.dma_start(dbg_h1[:, :], h1_bf[:])
                if DBG:
                    nc.sync.dma_start(dbg_x1[l, :, :], x_bf[:])

            # bf16 -> fp32 upcast on DVE, then a plain store
            x_f32 = acts.tile([128, DT * NS], F32)
            nc.vector.tensor_copy(x_f32[:], x_bf[:])
            nc.sync.dma_start(out_e[:, :], x_f32[:])

    nc.finalize()
    return nc


_CACHED = {}


def kernel(**inputs):
    inputs = {k: np.asarray(v) for k, v in inputs.items()}
    host = prepare_host_inputs(inputs)

    if "nc" not in _CACHED:
        _CACHED["nc"] = build_nc()
    nc = _CACHED["nc"]

    in_maps = []
    for r in range(R):
        m = {
            "x_in": shard_x(inputs["motion_feats"], r),
            "wq": host["wq"], "wk": host["wk"], "wv": host["wv"], "wm": host["wm"],
            "wp1": host["wp1"], "wp2": host["wp2"],
            "bq": host["bq"], "bm": host["bm"], "s1": host["s1"], "b1": host["b1"],
        }
        in_maps.append(m)

    res = run_bass_kernel_spmd(nc, in_maps, core_ids=list(range(R)))
    return unshard_out([res.results[r]["out"] for r in range(R)])
